# revision 31
# baseline (speedup 1.0000x reference)
"""Trainium2 Bass kernel for the soft-decision-tree ensemble problem.

Math (per reference):
  I = onehot(argmax_d T[e,n,:]) ; t = max_d T[e,n,:]
  u[b,en] = t[en] - x[b, argmax_d] ; s = floor(u)
  p[b,e,l] = prod_j (bit ? 1-s : s) over the leaf's 6 ancestors
  out = softmax(p @ L, axis=classes)

Strategy: data-parallel over the batch across 8 cores (1024 rows each),
T/L replicated. Per core:
  Prologue: T^T loaded [128d, 4dc, 1008en] via 4 parallel DMA queues;
    tmax via free-axis reduce + DRAM round-trip -> t_row; t split into 3
    exact bf16 pieces; t broadcast by a K=1 matmul; I^T = is_equal(T^T,
    t_bcast) in the selection layout.
  Selection u = fl(t - x_sel) exactly: PSUM accumulates the 3 bf16
    t-pieces (disjoint mantissas, exact sum) then 4 transpose-mode
    one-hot gathers of -x (exact fp32 routing; 2 cyc/row).
  Floor via the 1.5*2^23 RNE magic constant (ACT add) + 2 DVE ops.
  Tree products batched per 4-chunk group on DVE; transpose p on PE;
  final fp32 matmul; softmax with batched stride-0-broadcast normalize.
Program order: all x-transposes first, then sel/floor chunks with tree
groups interleaved, then stage-B PE work — keeps the in-order engine
FIFOs from cross-stalling.
"""
import os
import sys

for p in ("/opt/trn_rl_repo",):
    if p not in sys.path and os.path.isdir(p):
        sys.path.insert(0, p)

import numpy as np

import concourse.bass as bass
import concourse.tile as tile
from concourse import bacc, mybir
from concourse.bass_utils import run_bass_kernel_spmd

# problem constants (hardcoded per contract)
B, D = 8192, 512
E, NN, NL, C = 16, 63, 64, 100
DEPTH = 6
NCORES = 8
BC = B // NCORES          # rows per core = 1024
CH = BC // 128            # 128-row chunks per core = 8
EN = E * NN               # 1008
HALF = EN // 2            # 504
GRP = 4                   # chunks per tree/softmax group
BIG = 12582912.0          # 1.5*2^23: ulp=1 across [2^23,2^24), RNE round-to-int

F32 = mybir.dt.float32
BF16 = mybir.dt.bfloat16

SEL_TG = os.environ.get("KERNEL_SEL_TG", "1") == "1"      # transpose-gather selection
T_BF16 = os.environ.get("KERNEL_T_BF16", "1") == "1"      # t-row as 3 exact bf16 pieces
FLOOR_V = os.environ.get("KERNEL_FLOOR_V", "act")         # act | dve: engine for v=u+BIG
FUSED_FLOOR = os.environ.get("KERNEL_FUSED_FLOOR", "1") == "1"  # 1-op custom DVE floor
FLOOR_T = os.environ.get("KERNEL_FLOOR_T", "1") == "1"    # fold t-add into fused floor


def _register_dve_op(name, spec_body_ref):
    from concourse import dve_ops
    from concourse.dve_spec import lower, _has_src1
    from concourse.dve_uop import DveOpSpec

    for o in dve_ops.OPS:
        if o.name == name:
            return o
    spec = spec_body_ref
    shas = {}
    for ver in ("v3", "v4"):
        try:
            uops = lower(spec, ver=ver)
            shas[ver] = DveOpSpec(
                name=name, opcode=0, uops=uops, rd1_en=_has_src1(spec)
            ).sha(ver)
        except Exception:
            pass
    op = dve_ops.DveOp(name, spec, subdim=False, uops_sha=shas)
    dve_ops._SUB_OPCODE_FOR_NAME[name] = dve_ops._CUSTOM_DVE_ROW_BASE + len(dve_ops.OPS)
    dve_ops.OPS.append(op)
    dve_ops.CUSTOM_DVE_SPECS[name] = spec
    return op


def _fused_floor_op():
    """w = -floor(in0): v = in0 + C0; r = v - C0 (RNE round, C0 = 1.5*2^23);
    w = (r > in0) - r. Each uop stage rounds to fp32. HW-validated exact."""
    from concourse.dve_spec import Spec, Src0, C0

    _v = Src0 + C0
    _r = _v - C0

    def _ref(in0, in1, s0, s1, imm2):
        f32 = np.float32
        v = (in0.astype(f32) + f32(s0)).astype(f32)
        r = (v - f32(s0)).astype(f32)
        return ((r > in0).astype(f32) - r).astype(f32)

    return _register_dve_op("NEG_FLOOR_ANT", Spec(body=(_r > Src0) - _r, reference=_ref))


def _fused_floor_t_op():
    """w = -floor(in0 + in1): s = in0 + in1 (single exact fp32 add);
    v = s + C0; r = v - C0; w = (r > s) - r."""
    from concourse.dve_spec import Spec, Src0, Src1, C0

    _s = Src0 + Src1
    _v = _s + C0
    _r = _v - C0

    def _ref(in0, in1, s0, s1, imm2):
        f32 = np.float32
        in1 = np.asarray(in1).reshape(np.asarray(in0).shape)
        s = (in0.astype(f32) + in1.astype(f32)).astype(f32)
        v = (s + f32(s0)).astype(f32)
        r = (v - f32(s0)).astype(f32)
        return ((r > s).astype(f32) - r).astype(f32)

    return _register_dve_op(
        "NEG_FLOOR_T_ANT", Spec(body=(_r > _s) - _r, reference=_ref)
    )


def build_program():
    nc = bacc.Bacc(
        "TRN2",
        target_bir_lowering=False,
        debug=False,
        enable_asserts=False,
        num_devices=NCORES,
    )

    x_in = nc.dram_tensor("x", [BC, D], F32, kind="ExternalInput").ap()
    T_in = nc.dram_tensor("T", [E, NN, D], F32, kind="ExternalInput").ap()
    L_in = nc.dram_tensor("L", [E, NL, C], F32, kind="ExternalInput").ap()
    idf_in = nc.dram_tensor("idf", [128, 128], F32, kind="ExternalInput").ap()
    out_d = nc.dram_tensor("out", [BC, C], F32, kind="ExternalOutput").ap()
    t_scratch = nc.dram_tensor("t_scratch", [EN], F32).ap()

    dma_engines = [nc.sync, nc.scalar, nc.sync, nc.scalar]

    with tile.TileContext(nc) as tc:
        with (
            tc.tile_pool(name="const", bufs=1) as constp,
            tc.tile_pool(name="tproc", bufs=1) as tprocp,
            tc.tile_pool(name="xpool", bufs=3) as xp,
            tc.tile_pool(name="xtall", bufs=1) as xtp,
            tc.tile_pool(name="wpool", bufs=1) as wbig,
            tc.tile_pool(name="work", bufs=2) as wp,
            tc.tile_pool(name="tree", bufs=2) as treep,
            tc.tile_pool(name="soft", bufs=2) as softp,
            tc.tile_pool(name="psu", bufs=2, space="PSUM") as psu,
            tc.tile_pool(name="pst", bufs=2, space="PSUM") as pst,
            tc.tile_pool(name="psy", bufs=2, space="PSUM") as psy,
        ):
            # ---- constants ----
            idf = constp.tile([128, 128], F32)
            nc.sync.dma_start(idf[:], idf_in[:])
            ones = constp.tile([1, 128], F32)
            nc.vector.memset(ones[:], 1.0)

            out_v = out_d.rearrange("(k p) c -> p k c", p=128)
            x_v = x_in.rearrange("(k p) d -> p k d", p=128)

            # ---- all x loads + transposes up front (PE busy during prologue) ----
            xT_all = xtp.tile([128, CH, 4, 128], F32)
            for k in range(CH):
                x_k = xp.tile([128, D], F32, tag="x")
                nc.scalar.dma_start(x_k[:], x_v[:, k, :])
                tp = pst.tile([128, 4, 128], F32, tag="tp")
                for c in range(4):
                    nc.tensor.transpose(tp[:, c, :], x_k[:, c * 128:(c + 1) * 128], idf[:])
                nc.scalar.activation(
                    xT_all[:, k, :, :], tp[:], mybir.ActivationFunctionType.Copy,
                    scale=-1.0,
                )


            # ---- T processing ----
            # T_sb [126, 8, D] loaded in 2 pipelined halves; tmax via
            # free-axis reduce; t_row via DRAM round-trip; I in the [126,8,D]
            # layout via stride-0-broadcast is_equal, then PE-transposed into
            # the selection layout I_dT [128, 4, EN].
            T_flat = T_in.rearrange("e n d -> (e n) d")
            T_v = T_flat.rearrange("(t p) d -> p t d", p=126)
            T_sb = tprocp.tile([126, 8, D], F32)
            tmax = tprocp.tile([126, 8], F32)
            scr_v = t_scratch.rearrange("(t p) -> p t", p=126)
            for hh in range(2):
                ts = slice(4 * hh, 4 * hh + 4)
                nc.sync.dma_start(T_sb[:, ts, :], T_v[:, ts, :])
                nc.vector.tensor_reduce(
                    tmax[:, ts], T_sb[:, ts, :], axis=mybir.AxisListType.X,
                    op=mybir.AluOpType.max,
                )
                nc.scalar.dma_start(scr_v[:, ts], tmax[:, ts])
            t_row = constp.tile([1, EN], F32)
            nc.sync.dma_start(t_row[:1, :], t_scratch.rearrange("(o x) -> o x", o=1))
            # I in [126, 8, D]: is_equal against tmax broadcast along d
            I_sb = tprocp.tile([126, 8, D], F32)
            tmax3 = tmax[:].rearrange("p (t o) -> p t o", o=1)
            for hh in range(2):
                ts = slice(4 * hh, 4 * hh + 4)
                nc.vector.tensor_tensor(
                    I_sb[:, ts, :], T_sb[:, ts, :],
                    tmax3[:, ts, :].broadcast_to([126, 4, D]),
                    op=mybir.AluOpType.is_equal,
                )

            if FLOOR_T:
                # t broadcast tile in SBUF for the fused floor's second stream
                tmb_ps = psu.tile([128, 2, 512], F32, tag="u")
                for h in range(2):
                    nc.tensor.matmul(
                        tmb_ps[:, h, 0:HALF], lhsT=ones[:1, :],
                        rhs=t_row[:1, h * HALF:(h + 1) * HALF],
                        start=True, stop=True,
                    )
                tmb_sb = constp.tile([128, 2, HALF], F32)
                nc.scalar.activation(
                    tmb_sb[:], tmb_ps[:, :, 0:HALF], mybir.ActivationFunctionType.Copy
                )
            elif T_BF16:
                # exact 3-term bf16 split: t = hi + mid + lo (disjoint mantissas)
                ones_bf = constp.tile([1, 128], BF16)
                nc.scalar.activation(ones_bf[:], ones[:], mybir.ActivationFunctionType.Copy)
                t_hi = constp.tile([1, EN], BF16)
                nc.vector.tensor_copy(t_hi[:], t_row[:])
                r1 = constp.tile([1, EN], F32)
                nc.vector.tensor_tensor(r1[:], t_row[:], t_hi[:], op=mybir.AluOpType.subtract)
                t_mid = constp.tile([1, EN], BF16)
                nc.vector.tensor_copy(t_mid[:], r1[:])
                r2 = constp.tile([1, EN], F32)
                nc.vector.tensor_tensor(r2[:], r1[:], t_mid[:], op=mybir.AluOpType.subtract)
                t_lo = constp.tile([1, EN], BF16)
                nc.vector.tensor_copy(t_lo[:], r2[:])
                t_pieces = [t_hi, t_mid, t_lo]

            # I^T via PE transposes: I_dT[d, c, en] with en = t*126 + p
            I_dT = constp.tile([128, 4, EN], F32)
            for t in range(8):
                tpI = pst.tile([128, 4, 128], F32, tag="tp")
                for c in range(4):
                    nc.tensor.transpose(
                        tpI[:, c, 0:126], I_sb[:, t, c * 128:(c + 1) * 128],
                        idf[:126, :126],
                    )
                nc.scalar.activation(
                    I_dT[:, :, t * 126:(t + 1) * 126], tpI[:, :, 0:126],
                    mybir.ActivationFunctionType.Copy,
                )

            L_sb = constp.tile([128, CH, C], F32)
            nc.scalar.dma_start(
                L_sb[:], L_in.rearrange("e l c -> (e l) c").rearrange("(j p) c -> p j c", p=128)
            )

            # w = -floor(u) for all chunks, consumed group-wise by the tree
            w_all = wbig.tile([128, CH, 2, HALF], BF16)

            def sel_floor(k):
                u_ps = psu.tile([128, 2, 512], F32, tag="u")
                for h in range(2):
                    hs = slice(h * HALF, (h + 1) * HALF)
                    use_t_mm = not FLOOR_T
                    if use_t_mm and T_BF16:
                        for i, tpc in enumerate(t_pieces):
                            nc.tensor.matmul(
                                u_ps[:, h, 0:HALF], lhsT=ones_bf[:1, :],
                                rhs=tpc[:1, hs], start=(i == 0), stop=False,
                            )
                    for c in range(4):
                        nc.tensor.matmul(
                            u_ps[:, h, 0:HALF],
                            lhsT=xT_all[:, k, c, :],
                            rhs=I_dT[:, c, hs],
                            is_transpose=SEL_TG,
                            start=(c == 0 if not use_t_mm else (T_BF16 is False and c == 0)),
                            stop=(c == 3 if not use_t_mm else (T_BF16 and c == 3)),
                        )
                    if use_t_mm and not T_BF16:
                        nc.tensor.matmul(
                            u_ps[:, h, 0:HALF], lhsT=ones[:1, :],
                            rhs=t_row[:1, hs], start=False, stop=True,
                        )
                # floor: w = (v-BIG > u) + BIG - v, v = RNE-round via +BIG
                if FLOOR_T:
                    nc.vector._custom_dve(
                        _fused_floor_t_op(), out=w_all[:, k, :, :],
                        in0=u_ps[:, :, 0:HALF], in1=tmb_sb[:], s0=BIG,
                    )
                    return
                if FUSED_FLOOR:
                    nc.vector._custom_dve(
                        _fused_floor_op(), out=w_all[:, k, :, :],
                        in0=u_ps[:, :, 0:HALF], s0=BIG,
                    )
                    return
                v_sb = wp.tile([128, 2, HALF], F32, tag="v")
                if FLOOR_V == "act":
                    nc.scalar.activation(
                        v_sb[:], u_ps[:, :, 0:HALF],
                        mybir.ActivationFunctionType.Copy, bias=BIG,
                    )
                else:
                    nc.vector.tensor_scalar(
                        v_sb[:], u_ps[:, :, 0:HALF], BIG, None, op0=mybir.AluOpType.add
                    )
                flag = wp.tile([128, 2, HALF], F32, tag="fl")
                nc.vector.scalar_tensor_tensor(
                    flag[:], v_sb[:], -BIG, u_ps[:, :, 0:HALF],
                    op0=mybir.AluOpType.add, op1=mybir.AluOpType.is_gt,
                )
                nc.vector.scalar_tensor_tensor(
                    w_all[:, k, :, :], flag[:], BIG, v_sb[:],
                    op0=mybir.AluOpType.add, op1=mybir.AluOpType.subtract,
                )

            tree_out = []

            def tree_group(g):
                ks = g * GRP
                w4 = w_all[:, ks:ks + GRP, :, :].rearrange(
                    "p k h q -> p k (h q)"
                ).rearrange("p k (e n) -> p k e n", n=NN)
                lvl = treep.tile([128, GRP, E, 2], F32, tag="lvlA")
                nc.vector.tensor_scalar(
                    lvl[:, :, :, 0:1], w4[:, :, :, 0:1], -1.0, None,
                    op0=mybir.AluOpType.mult,
                )
                nc.vector.tensor_scalar(
                    lvl[:, :, :, 1:2], w4[:, :, :, 0:1], 1.0, None,
                    op0=mybir.AluOpType.add,
                )
                for j in range(2, DEPTH + 1):
                    half = 2 ** (j - 1)
                    base = half - 1
                    nxt = treep.tile(
                        [128, GRP, E, 2 * half], F32, tag=("lvlA" if j % 2 else "lvlB")
                    )
                    nxt5 = nxt[:].rearrange("p k e (k2 c) -> p k e k2 c", c=2)
                    wj = w4[:, :, :, base:base + half]
                    par = lvl[:]
                    nc.vector.scalar_tensor_tensor(
                        nxt5[:, :, :, :, 0], wj, -1.0, par,
                        op0=mybir.AluOpType.mult, op1=mybir.AluOpType.mult,
                    )
                    nc.vector.scalar_tensor_tensor(
                        nxt5[:, :, :, :, 1], wj, 1.0, par,
                        op0=mybir.AluOpType.add, op1=mybir.AluOpType.mult,
                    )
                    lvl = nxt
                tree_out.append(lvl)  # [128, GRP, E, NL]

            def stage_b(g):
                ks = g * GRP
                p_g = tree_out[g]
                y_ps = psy.tile([128, GRP, 128], F32, tag="y")
                ssum = softp.tile([128, GRP], F32, tag="ss")
                nmx = softp.tile([128, GRP], F32, tag="nm")
                yexp = softp.tile([128, GRP, C], F32, tag="ye")
                for kk in range(GRP):
                    p_flat = p_g[:, kk, :, :].rearrange("p e l -> p (e l)")
                    pT_k = wp.tile([128, CH, 128], F32, tag="pT")
                    for jh in range(2):
                        tp2 = pst.tile([128, 4, 128], F32, tag="tp")
                        for c in range(4):
                            j = jh * 4 + c
                            nc.tensor.transpose(
                                tp2[:, c, :], p_flat[:, j * 128:(j + 1) * 128], idf[:]
                            )
                        nc.scalar.activation(
                            pT_k[:, jh * 4:(jh + 1) * 4, :], tp2[:],
                            mybir.ActivationFunctionType.Copy,
                        )
                    for j in range(CH):
                        nc.tensor.matmul(
                            y_ps[:, kk, 0:C],
                            lhsT=pT_k[:, j, :],
                            rhs=L_sb[:, j, :],
                            start=(j == 0), stop=(j == CH - 1),
                        )
                    nc.vector.tensor_reduce(
                        nmx[:, kk:kk + 1], y_ps[:, kk, 0:C], axis=mybir.AxisListType.X,
                        op=mybir.AluOpType.max, negate=True,
                    )
                    nc.scalar.activation(
                        yexp[:, kk, :], y_ps[:, kk, 0:C],
                        mybir.ActivationFunctionType.Exp,
                        bias=nmx[:, kk:kk + 1], scale=1.0,
                        accum_out=ssum[:, kk:kk + 1],
                    )
                rec = softp.tile([128, GRP], F32, tag="rc")
                nc.vector.reciprocal(rec[:], ssum[:])
                yout = softp.tile([128, GRP, C], F32, tag="yo")
                rec3 = rec[:].rearrange("p (f o) -> p f o", o=1)
                nc.vector.tensor_tensor(
                    yout[:], yexp[:], rec3.broadcast_to([128, GRP, C]),
                    op=mybir.AluOpType.mult,
                )
                nc.sync.dma_start(out_v[:, ks:ks + GRP, :], yout[:])

            # emission order: sel/floor(0..3), tree(g0), sel/floor(4..7),
            # tree(g1), stage_b(g0), stage_b(g1)
            for k in range(GRP):
                sel_floor(k)
            tree_group(0)
            for k in range(GRP, CH):
                sel_floor(k)
            tree_group(1)
            stage_b(0)
            stage_b(1)

    nc.compile()
    return nc


_id_f32 = np.eye(128, dtype=np.float32)


def make_in_maps(x, T, L):
    x = np.ascontiguousarray(x, dtype=np.float32)
    T = np.ascontiguousarray(T, dtype=np.float32)
    L = np.ascontiguousarray(L, dtype=np.float32)
    maps = []
    for i in range(NCORES):
        maps.append({
            "x": x[i * BC:(i + 1) * BC],
            "T": T,
            "L": L,
            "idf": _id_f32,
        })
    return maps


def run(x, T, L, trace=False, **kw):
    nc = build_program()
    res = run_bass_kernel_spmd(
        nc, make_in_maps(x, T, L), core_ids=list(range(NCORES)), trace=trace, **kw
    )
    out = np.concatenate([res.results[i]["out"] for i in range(NCORES)], axis=0)
    return out, res


def kernel(x, T, L):
    out, _ = run(x, T, L, trace=False)
    return out
